# revision 1
# baseline (speedup 1.0000x reference)
"""Trainium2 Bass kernel for grouped-query causal self-attention.

Problem shapes (hardcoded): x [8,1024,1024] f32, W_attn [6144,1024] f32,
W_proj [1024,4096] f32. 16 heads, head_dim 64, 4 query sets sharing one K/V.

Sharding: data parallel over batch — one batch element per NeuronCore (8 cores).
No collectives needed.

Per-core algorithm (everything "transposed" = [feature, token] layout so no
on-device transposes are needed; x is pre-transposed on the host):
  1. qkvT tiles = W_attn @ x^T   (stationary = W_attn^T tile, moving = x^T)
     -> K^T [1024f, 1024t], Q_g^T per set, and V in normal [t, f] layout
        (V's matmul uses x^T tiles as stationary instead).
  2. Attention per (set g, head h), computed transposed, 512-wide q chunks:
        S^T[k, q] = K_tile^T-stationary @ Q^T-moving   (contraction = head_dim)
        P^T = exp(S^T * scale)        (no max subtraction needed: logits ~N(0,1))
        causal: trim q-range per k-tile; zero the 128x128 diagonal triangle of
        P^T via gpsimd affine_select (parallel engine, off the PE)
        y_aug^T[d, q] = V_aug-stationary @ P^T-moving  (V augmented with a ones
        column -> row 64 of y_aug^T = softmax denominator, for free)
        normalize: denominators bounce through DRAM reshaped to 8 lanes for the
        exact DVE reciprocal, then partition-broadcast back and multiply.
  3. out = combined @ W_proj^T accumulated over sets (stationary = y^T tiles,
     moving = W_proj^T streamed from DRAM).
dtypes: bf16 operands for matmuls (fp32 PSUM accumulate), fp32 softmax
denominator path and output accumulation.
"""

import math

import ml_dtypes
import numpy as np

import concourse.bacc as bacc
import concourse.bass as bass
import concourse.mybir as mybir
import concourse.tile as tile
from concourse.bass_utils import run_bass_kernel_spmd

BF16 = ml_dtypes.bfloat16

B, T, C = 8, 1024, 1024
NH, HD, NQS = 16, 64, 4
SCALE = 1.0 / math.sqrt(HD)
NT = T // 128  # token tiles
NCH = C // 128  # channel tiles
KOFF = NQS * C  # 4096: K rows in W_attn
VOFF = (NQS + 1) * C  # 5120: V rows in W_attn

_CACHE = {}
LAST = {}  # exec_time_ns etc for test harness


def _build():
    f32 = mybir.dt.float32
    bf16 = mybir.dt.bfloat16
    EXP = mybir.ActivationFunctionType.Exp

    nc = bacc.Bacc()
    xT = nc.declare_dram_parameter("xT", [C, T], bf16, isOutput=False)
    waT = nc.declare_dram_parameter("waT", [C, 6 * C], bf16, isOutput=False)
    wpT = nc.declare_dram_parameter("wpT", [NQS * C, C], bf16, isOutput=False)
    vonesD = nc.declare_dram_parameter("vones", [128, NH, 1], bf16, isOutput=False)
    identD = nc.declare_dram_parameter("ident", [128, 128], bf16, isOutput=False)
    cmaskD = nc.declare_dram_parameter("cmaskT", [128, 128], bf16, isOutput=False)
    out = nc.declare_dram_parameter("out", [T, C], f32, isOutput=True)
    # DRAM bounce rows: denominators (rows 0-127) and their reciprocals
    # (rows 128-255); reciprocal rows are read back partition-broadcast.
    rscratch = nc.dram_tensor("rscratch", [2 * NQS * NH * 2, 512], f32)

    with tile.TileContext(nc) as tc:
        with (
            tc.tile_pool(name="res", bufs=1) as res,
            tc.tile_pool(name="wa", bufs=16) as wa_pool,
            tc.tile_pool(name="wp", bufs=16) as wp_pool,
            tc.tile_pool(name="pt", bufs=6) as pt_pool,
            tc.tile_pool(name="yab", bufs=4) as yab_pool,
            tc.tile_pool(name="small", bufs=4) as small_pool,
            tc.tile_pool(name="pacc", bufs=4, space="PSUM") as pacc,
            tc.tile_pool(name="ptmp", bufs=4, space="PSUM") as ptmp,
        ):
            xt = [res.tile([128, T], bf16, tag=f"xt{i}", name=f"xt{i}") for i in range(NCH)]
            kt = [res.tile([128, T], bf16, tag=f"kt{i}", name=f"kt{i}") for i in range(NCH)]
            vt = [res.tile([128, NH, HD + 1], bf16, tag=f"vt{i}", name=f"vt{i}") for i in range(NT)]
            qt = [res.tile([128, T], bf16, tag=f"qt{i}", name=f"qt{i}") for i in range(NCH)]
            yt = [res.tile([128, T], bf16, tag=f"yt{i}", name=f"yt{i}") for i in range(NCH)]
            osb = [res.tile([128, C], f32, tag=f"osb{i}", name=f"osb{i}") for i in range(NT)]

            ident = res.tile([128, 128], bf16, tag="ident", name="ident")
            cmask = res.tile([128, 128], bf16, tag="cmask", name="cmask")
            nc.sync.dma_start(out=ident, in_=identD[:, :])
            nc.sync.dma_start(out=cmask, in_=cmaskD[:, :])
            for tt in range(NT):
                nc.sync.dma_start(out=vt[tt][:, :, HD : HD + 1], in_=vonesD[:, :, :])

            for i in range(NCH):
                nc.sync.dma_start(out=xt[i], in_=xT[i * 128 : (i + 1) * 128, :])

            def project_T(dst, fbase, tag):
                """dst[i][f_local, t] = (x @ W_attn.T).T rows fbase..fbase+1024."""
                for fg in range(2):  # 512-wide feature groups
                    was = []
                    for ct in range(NCH):
                        w = wa_pool.tile(
                            [128, 512], bf16, tag="wa", name=f"wa_{tag}_{fg}_{ct}"
                        )
                        f0 = fbase + fg * 512
                        nc.sync.dma_start(
                            out=w, in_=waT[ct * 128 : (ct + 1) * 128, f0 : f0 + 512]
                        )
                        was.append(w)
                    for tc2 in range(2):
                        for ftl in range(4):
                            ps = pacc.tile(
                                [128, 512], f32, tag="pacc",
                                name=f"ps_{tag}_{fg}_{tc2}_{ftl}",
                            )
                            for ct in range(NCH):
                                nc.tensor.matmul(
                                    ps,
                                    was[ct][:, ftl * 128 : (ftl + 1) * 128],
                                    xt[ct][:, tc2 * 512 : (tc2 + 1) * 512],
                                    start=(ct == 0),
                                    stop=(ct == NCH - 1),
                                )
                            fti = fg * 4 + ftl
                            nc.vector.tensor_copy(
                                dst[fti][:, tc2 * 512 : (tc2 + 1) * 512], ps
                            )

            project_T(kt, KOFF, "k")

            # V in [token, feature] layout, features interleaved with a ones
            # column every 64 (each head's stationary V_aug slice is [128, 65]).
            for fg in range(2):
                was = []
                for ct in range(NCH):
                    w = wa_pool.tile([128, 512], bf16, tag="wa", name=f"wav_{fg}_{ct}")
                    f0 = VOFF + fg * 512
                    nc.sync.dma_start(
                        out=w, in_=waT[ct * 128 : (ct + 1) * 128, f0 : f0 + 512]
                    )
                    was.append(w)
                for tt in range(NT):
                    ps = pacc.tile([128, 512], f32, tag="pacc", name=f"psv_{fg}_{tt}")
                    for ct in range(NCH):
                        nc.tensor.matmul(
                            ps,
                            xt[ct][:, tt * 128 : (tt + 1) * 128],
                            was[ct],
                            start=(ct == 0),
                            stop=(ct == NCH - 1),
                        )
                    nc.vector.tensor_copy(
                        vt[tt][:, fg * 8 : (fg + 1) * 8, 0:HD],
                        ps.rearrange("p (a b) -> p a b", b=HD),
                    )

            for g in range(NQS):
                project_T(qt, g * C, f"q{g}")

                for h in range(NH):
                    ft, ro = h // 2, (h % 2) * 64
                    for qc in range(2):  # 512-wide query chunks
                        yp = pacc.tile(
                            [128, 512], f32, tag="pacc", name=f"yp{g}_{h}_{qc}"
                        )
                        nkt = 4 * qc + 4
                        for k2 in range(nkt):
                            qlo = max(qc * 512, k2 * 128)
                            w = qc * 512 + 512 - qlo
                            sp = ptmp.tile(
                                [128, 512], f32, tag="ptmp",
                                name=f"sp{g}_{h}_{qc}_{k2}",
                            )
                            diag = k2 * 128 >= qc * 512
                            nc.tensor.matmul(
                                sp[:, :w],
                                kt[ft][ro : ro + 64, k2 * 128 : (k2 + 1) * 128],
                                qt[ft][ro : ro + 64, qlo : qlo + w],
                                start=True,
                                stop=not diag,
                            )
                            if diag:
                                # additive causal mask (0 / -1e30) on the
                                # diagonal 128x128 block, applied on the PE so
                                # exp/AV deps stay single-engine
                                nc.tensor.matmul(
                                    sp[:, 0:128],
                                    cmask,
                                    ident,
                                    start=False,
                                    stop=True,
                                    skip_group_check=True,
                                )
                            pt = pt_pool.tile(
                                [128, 512], bf16, tag="pt",
                                name=f"pt{g}_{h}_{qc}_{k2}",
                            )
                            nc.scalar.activation(
                                pt[:, :w], sp[:, :w], EXP, bias=0.0, scale=SCALE
                            )
                            off = qlo - qc * 512
                            nc.tensor.matmul(
                                yp[0:65, off : off + w],
                                vt[k2][:, h, :],
                                pt[:, :w],
                                start=(k2 == 0),
                                stop=(k2 == nkt - 1),
                            )
                        yab = yab_pool.tile(
                            [65, 512], f32, tag="yab", name=f"yab{g}_{h}_{qc}"
                        )
                        nc.vector.tensor_copy(yab, yp[0:65, :])
                        # bounce the single-partition denominator row through
                        # DRAM reshaped to 8 lanes, so the exact DVE
                        # reciprocal isn't single-lane (3.3us -> 0.3us)
                        ridx = (g * NH + h) * 2 + qc
                        drow = rscratch[ridx : ridx + 1, :]
                        nc.sync.dma_start(out=drow, in_=yab[64:65, :])
                        den8 = small_pool.tile(
                            [8, 64], f32, tag="den8", name=f"den8{g}_{h}_{qc}"
                        )
                        nc.sync.dma_start(
                            out=den8, in_=drow.rearrange("a (b c) -> (a b) c", b=8)
                        )
                        rec8 = small_pool.tile(
                            [8, 64], f32, tag="rec8", name=f"rec8{g}_{h}_{qc}"
                        )
                        nc.vector.reciprocal(rec8, den8)
                        rrow = rscratch[128 + ridx : 128 + ridx + 1, :]
                        nc.sync.dma_start(
                            out=rrow.rearrange("a (b c) -> (a b) c", b=8), in_=rec8
                        )
                        bcst = small_pool.tile(
                            [64, 512], f32, tag="bcst", name=f"bcst{g}_{h}_{qc}"
                        )
                        nc.sync.dma_start(
                            out=bcst,
                            in_=bass.AP(
                                tensor=rrow.tensor,
                                offset=rrow.offset,
                                ap=[[0, 64]] + rrow.ap[1:],
                            ),
                        )
                        nc.vector.tensor_mul(
                            yt[ft][ro : ro + 64, qc * 512 : qc * 512 + 512],
                            yab[0:64, :],
                            bcst,
                        )

                # projection for this set, accumulated into osb
                for cc in range(2):
                    wps = []
                    for ftl in range(NCH):
                        wpt = wp_pool.tile(
                            [128, 512], bf16, tag="wp", name=f"wp{g}_{cc}_{ftl}"
                        )
                        nc.sync.dma_start(
                            out=wpt,
                            in_=wpT[
                                g * C + ftl * 128 : g * C + (ftl + 1) * 128,
                                cc * 512 : (cc + 1) * 512,
                            ],
                        )
                        wps.append(wpt)
                    for tt in range(NT):
                        ps = pacc.tile(
                            [128, 512], f32, tag="pacc", name=f"psp{g}_{cc}_{tt}"
                        )
                        for ftl in range(NCH):
                            nc.tensor.matmul(
                                ps,
                                yt[ftl][:, tt * 128 : (tt + 1) * 128],
                                wps[ftl],
                                start=(ftl == 0),
                                stop=(ftl == NCH - 1),
                            )
                        dst = osb[tt][:, cc * 512 : (cc + 1) * 512]
                        if g == 0:
                            nc.vector.tensor_copy(dst, ps)
                        else:
                            nc.vector.tensor_add(dst, dst, ps)

            for tt in range(NT):
                nc.sync.dma_start(out=out[tt * 128 : (tt + 1) * 128, :], in_=osb[tt])

    nc.compile()
    return nc


def kernel(x, W_attn, W_proj, _trace=False):
    if "nc" not in _CACHE:
        _CACHE["nc"] = _build()
    nc = _CACHE["nc"]

    xT = np.ascontiguousarray(np.transpose(np.asarray(x, np.float32), (0, 2, 1))).astype(BF16)
    waT = np.ascontiguousarray(np.asarray(W_attn, np.float32).T).astype(BF16)
    wpT = np.ascontiguousarray(np.asarray(W_proj, np.float32).T).astype(BF16)
    vones = np.ones((128, NH, 1), np.float32).astype(BF16)
    ii = np.arange(128)
    ident = np.eye(128, dtype=np.float32).astype(BF16)
    # lhsT for the mask matmul: out[k,q] = cmaskT[q,k] = 0 if q>=k else -1e30
    cmaskT = (
        np.where(ii[:, None] >= ii[None, :], 0.0, -1e30)
        .astype(np.float32)
        .astype(BF16)
    )

    in_maps = [
        {"xT": xT[b], "waT": waT, "wpT": wpT, "vones": vones, "ident": ident,
         "cmaskT": cmaskT}
        for b in range(B)
    ]
    res = run_bass_kernel_spmd(nc, in_maps, core_ids=list(range(B)), trace=_trace)
    LAST["exec_time_ns"] = res.exec_time_ns
    LAST["mean_exec_time_ns"] = res.mean_exec_time_ns
    LAST["results"] = res
    return np.stack([res.results[b]["out"] for b in range(B)]).astype(np.float32)

